# revision 5
# baseline (speedup 1.0000x reference)
"""NT-Xent loss on 8 Trainium2 NeuronCores — v2.

Baseline (117us) was serialized on two ~55us rooflines: bf16 matmuls on PE
and exp on the ACT engine (1 elem/cycle/lane). v2 attacks both:

  1. fp8e4 DoubleRow matmuls: contract-256 in ONE instruction at ~2x bf16
     stream rate. znT8 [128, 2, 8192] fp8 built on-device; the bf16->fp8
     cast rides a gpsimd (SWDGE) casting DMA — zero compute-engine time.
  2. exp is split between ACT (exact spline exp, accum_out rowsum) and a
     custom fused DVE op EXP8_SUM: Schraudolph-style fast exp done shift-free
     as an 8th power — v = f32(s*C0 + C1) captures round(t*2^20) in the
     2^23 binade (value==bits there), u = v*2^-23 slides the exponent field
     exactly, u^8 = 16*exp(2s)*(1+eps), eps ~ +-30% pointwise sawtooth but
     the C1 tuning zeroes the exp-weighted mean: denominator-sum error
     ~2e-3, loss error ~1e-4 (gate is 2e-2). accum=add fuses the rowsum.
     The stray 16x is divided out in the tail.
  3. Window split: per m-chunk, 4 column windows of 2048; engines assigned
     ~22 ACT / 10 DVE so both run flat out while DVE also owns phase 1
     (sumsq via stt, Newton rsqrt, zn=z*r) which hides under the all-ACT q0.

Per core c: host rotates z by -1024c rows (identical SPMD program); rows
0..1023 are local; positives at +4096; diag handled as constant e^2.
Host sums the 8 scalar partials / 8192.
"""

import os
import sys

sys.path.insert(0, "/opt/trn_rl_repo")
os.environ.setdefault("MYCRO_LOCAL_CACHE", "1")

from operator import add as _op_add

import numpy as np

import concourse.bass as bass
import concourse.mybir as mybir
from concourse import bacc, tile
from concourse.bass_utils import run_bass_kernel_spmd

F32 = mybir.dt.float32
BF16 = mybir.dt.bfloat16
FP8 = mybir.dt.float8e4
AF = mybir.ActivationFunctionType
ALU = mybir.AluOpType
DR = mybir.MatmulPerfMode.DoubleRow

N_CORES = 8
TWO_N = 8192
D = 256
P = 128
NCHUNK = TWO_N // P               # 64 row-chunks of 128
NGROUPS = 8                       # phase-1 pipeline groups
GCHUNK = NCHUNK // NGROUPS        # 8 chunks per group
GROWS = TWO_N // NGROUPS          # 1024 rows per group
ROWS_PER_CORE = TWO_N // N_CORES  # 1024
M_CHUNKS = ROWS_PER_CORE // P     # 8 local row chunks
NCOL = 512                        # matmul free dim (one PSUM bank)
QCOL = 2048                       # consumer window = 4 banks
N_Q = TWO_N // QCOL               # 4
POS_OFF = TWO_N // 2              # 4096
TEMP_SCALE = 2.0                  # 1 / temperature

# ---- EXP8 fast-exp constants (calibrated; see module docstring) ----
_LOG2E = float(np.log2(np.e))
EXP8_C0 = float(np.float32(2.0 * _LOG2E / 8.0 * 2.0**23))
EXP8_C1 = float(np.float32(1.5 * 2.0**23 - 721420.3))
EXP8_C2 = float(np.float32(2.0**-23))
EXP8_SCALE = 16.0                 # u^8 = 16*exp(2s); divided out in tail

# window -> engine: 'A' = ACT exact exp, 'D' = DVE EXP8. q0 all-ACT (DVE
# still finishing phase 1); later qs alternate so both engines stay busy.
_DPAT = {1: (0, 2, 4, 6), 2: (1, 3, 5, 7), 3: (2, 5)}


def _win_engine(m, q):
    return "D" if m in _DPAT.get(q, ()) else "A"


# ---- custom DVE op registration ----
EXP8_NAME = "EXP8_SUM_ANT"


def _exp8_reference(in0, in1, c0, c1, c2):
    p = in0.shape[0]
    v = (in0.astype(np.float32) * np.float32(c0)).astype(np.float32)
    v = (v + np.float32(c1)).astype(np.float32)
    u = (v * np.float32(c2)).astype(np.float32)
    u2 = (u * u).astype(np.float32)
    u4 = (u2 * u2).astype(np.float32)
    u8 = (u4 * u4).astype(np.float32)
    return u8, u8.reshape(p, -1).sum(axis=-1, keepdims=True).astype(np.float32)


def _register_exp8():
    import concourse.dve_ops as dve_ops
    from concourse.dve_spec import C0, C1, C2, Spec, Src0, Zero, _has_src1, lower, sq
    from concourse.dve_uop import DveOpSpec

    for op in dve_ops.OPS:
        if op.name == EXP8_NAME:
            return op
    spec = Spec(
        body=sq(sq(sq((Src0 * C0 + C1) * C2))),
        accum=_op_add,
        accum_init=Zero,
        reference=_exp8_reference,
    )
    row = dve_ops._CUSTOM_DVE_ROW_BASE + len(dve_ops.OPS)
    assert row < 0x20, "custom DVE opcode rows exhausted"
    dve_ops._SUB_OPCODE_FOR_NAME[EXP8_NAME] = row
    shas = {}
    for ver in ("v3", "v4"):
        try:
            lowered = DveOpSpec(
                name=EXP8_NAME, opcode=row, uops=lower(spec, ver=ver),
                rd1_en=_has_src1(spec),
            )
            shas[ver] = lowered.sha(ver)
        except Exception:
            if ver == "v3":
                raise
    op = dve_ops.DveOp(EXP8_NAME, spec, subdim=False, uops_sha=shas)
    dve_ops.OPS.append(op)
    dve_ops.CUSTOM_DVE_SPECS[EXP8_NAME] = spec
    return op


EXP8_OP = _register_exp8()

_NC_CACHE = {}


def _build_nc():
    nc = bacc.Bacc(
        "TRN2",
        target_bir_lowering=False,
        debug=False,
        enable_asserts=False,
        num_devices=N_CORES,
    )
    z = nc.dram_tensor("z", [TWO_N, D], BF16, kind="ExternalInput")
    ident = nc.dram_tensor("ident", [P, P], F32, kind="ExternalInput")
    ones = nc.dram_tensor("ones", [P, 1], F32, kind="ExternalInput")
    out = nc.dram_tensor("partial", [1, 1], F32, kind="ExternalOutput")
    zs = nc.dram_tensor("zscratch", [TWO_N, D], BF16, kind="Internal")

    with tile.TileContext(nc) as tc:
        with (
            tc.tile_pool(name="big", bufs=1) as big,
            tc.tile_pool(name="zbpool", bufs=8) as zbpool,
            tc.tile_pool(name="znpool", bufs=4) as znpool,
            tc.tile_pool(name="work", bufs=2) as work,
        ):
            id_sb = big.tile([P, P], F32)
            nc.sync.dma_start(id_sb[:], ident[:])
            ones_sb = big.tile([P, 1], F32)
            nc.sync.dma_start(ones_sb[:], ones[:])

            zt0 = big.tile([P, TWO_N], BF16)
            zt1 = big.tile([P, TWO_N], BF16)
            zts = [zt0, zt1]
            znT8 = big.tile([P, 2, TWO_N], FP8)
            ss = big.tile([P, NCHUNK], F32)
            inv = big.tile([P, NCHUNK], F32)
            sums_a = big.tile([P, M_CHUNKS, N_Q], F32)
            sums_d = big.tile([P, M_CHUNKS, N_Q], F32)
            nc.gpsimd.memset(sums_a[:], 0.0)
            nc.gpsimd.memset(sums_d[:], 0.0)
            pos = big.tile([P, M_CHUNKS], F32)

            zv = z[:].rearrange("(n p) d -> p n d", p=P)
            zsv = zs[:].rearrange("(n p) d -> p n d", p=P)

            # all loads issued up front on the scalar-engine HWDGE ring
            zbg = []
            for g in range(NGROUPS):
                zb = zbpool.tile([P, GCHUNK, D], BF16, tag="zb")
                nc.scalar.dma_start(zb[:], zv[:, g * GCHUNK:(g + 1) * GCHUNK, :])
                zbg.append(zb)

            def sumsq_group(g):
                for c in range(GCHUNK):
                    scr = work.tile([P, D], BF16, tag="sqscr")
                    col = g * GCHUNK + c
                    nc.vector.scalar_tensor_tensor(
                        out=scr[:], in0=zbg[g][:, c, :], scalar=1.0,
                        in1=zbg[g][:, c, :], op0=ALU.mult, op1=ALU.mult,
                        accum_out=ss[:, col:col + 1])

            def newton_inv(lo, hi):
                """inv[:, lo:hi] = rsqrt(ss[:, lo:hi]) via 2 Newton steps."""
                w = hi - lo
                rd = 1.0 / float(np.sqrt(D))
                s = ss[:, lo:hi]
                ya = work.tile([P, w], F32, tag="ny0", bufs=2)
                nc.vector.tensor_scalar(ya[:], s, -0.5 / D * rd, 1.5 * rd,
                                        ALU.mult, ALU.add)
                for it in range(2):
                    t1 = work.tile([P, w], F32, tag=f"nt1_{it}", bufs=2)
                    nc.vector.tensor_mul(t1[:], ya[:], ya[:])
                    t2 = work.tile([P, w], F32, tag=f"nt2_{it}", bufs=2)
                    nc.vector.tensor_mul(t2[:], t1[:], s)
                    t3 = work.tile([P, w], F32, tag=f"nt3_{it}", bufs=2)
                    nc.vector.tensor_scalar(t3[:], t2[:], -0.5, 1.5,
                                            ALU.mult, ALU.add)
                    if it == 1:
                        nc.vector.tensor_mul(inv[:, lo:hi], ya[:], t3[:])
                    else:
                        yb = work.tile([P, w], F32, tag=f"ny{it + 1}", bufs=2)
                        nc.vector.tensor_mul(yb[:], ya[:], t3[:])
                        ya = yb

            def finish_group(g):
                """zn = z * inv (DVE), store, transpose, fp8-cast via SWDGE.
                Transposes alternate between the sync and scalar HWDGE rings
                so consecutive groups' transposes overlap."""
                znb = znpool.tile([P, GCHUNK, D], BF16, tag="znb")
                for c in range(GCHUNK):
                    col = g * GCHUNK + c
                    nc.vector.tensor_scalar_mul(
                        znb[:, c, :], zbg[g][:, c, :], inv[:, col:col + 1])
                sl = slice(g * GCHUNK, (g + 1) * GCHUNK)
                nc.sync.dma_start(zsv[:, sl, :], znb[:])
                rsl = slice(g * GROWS, (g + 1) * GROWS)
                nc.sync.dma_start_transpose(zt0[:, rsl], zs[rsl, 0:P])
                nc.scalar.dma_start_transpose(zt1[:, rsl], zs[rsl, P:2 * P])
                # bf16 -> fp8e4 dtype-casting DMA (only legal from the
                # gpsimd/SWDGE queue); zero compute-engine cost.
                nc.gpsimd.dma_start(znT8[:, 0, rsl], zt0[:, rsl])
                nc.gpsimd.dma_start(znT8[:, 1, rsl], zt1[:, rsl])
                return znb

            def main_win(m, q):
                """One [128, 2048] window: 4 fp8 DoubleRow matmuls
                (contract 256 in one instruction each), then exp+rowsum on
                ACT (exact) or DVE (EXP8 fast-exp, 16x-scaled)."""
                pt = psum_pool.tile([P, QCOL], F32, tag="sim")
                lhsT = znT8[:, :, m * P:(m + 1) * P]
                col0 = q * QCOL
                for nn in range(QCOL // NCOL):
                    col = col0 + nn * NCOL
                    nc.tensor.matmul(
                        pt[:, nn * NCOL:(nn + 1) * NCOL],
                        lhsT,
                        znT8[:, :, col:col + NCOL],
                        start=True, stop=True, perf_mode=DR,
                    )
                pcol = POS_OFF + m * P
                if col0 <= pcol < col0 + QCOL:
                    off = pcol - col0
                    scr = work.tile([P, P], F32, tag="extr")
                    nc.vector.scalar_tensor_tensor(
                        out=scr[:], in0=pt[:, off:off + P],
                        scalar=1.0, in1=id_sb[:],
                        op0=ALU.mult, op1=ALU.mult,
                        accum_out=pos[:, m:m + 1])
                if _win_engine(m, q) == "A":
                    nc.scalar.activation(
                        pt[:], pt[:], AF.Exp, scale=TEMP_SCALE,
                        accum_out=sums_a[:, m, q:q + 1])
                else:
                    scr8 = work.tile([P, QCOL], BF16, tag="e8out", bufs=2)
                    nc.vector._custom_dve(
                        EXP8_OP, out=scr8[:], in0=pt[:],
                        s0=EXP8_C0, s1=EXP8_C1, imm2=EXP8_C2,
                        accum_out=sums_d[:, m, q:q + 1])

            with tc.tile_pool(name="psum", bufs=2, space="PSUM") as psum_pool:
                # phase 1 fully front-loaded, per-group chains so the stages
                # pipeline across groups on different engines/queues
                znb0 = None
                for g in range(NGROUPS):
                    sumsq_group(g)
                    newton_inv(g * GCHUNK, (g + 1) * GCHUNK)
                    znb = finish_group(g)
                    if g == 0:
                        znb0 = znb
                        # warm-up matmuls bridge the PE HAM window
                        ptw = psum_pool.tile([P, QCOL], F32, tag="sim")
                        for j in range(8):
                            nc.tensor.matmul(
                                ptw[:, (j % 4) * NCOL:(j % 4) * NCOL + NCOL],
                                znb0[:, 0, 0:P],
                                znb0[:, 2 * (j % 4):2 * (j % 4) + 2, :],
                                start=True, stop=True, skip_group_check=True)
                for q in range(N_Q):
                    for m in range(M_CHUNKS):
                        main_win(m, q)

            # ---- tail: per-core partial loss ----
            ra = big.tile([P, M_CHUNKS], F32)
            nc.vector.tensor_reduce(ra[:], sums_a[:],
                                    axis=mybir.AxisListType.X, op=ALU.add)
            rdv = big.tile([P, M_CHUNKS], F32)
            nc.vector.tensor_reduce(rdv[:], sums_d[:],
                                    axis=mybir.AxisListType.X, op=ALU.add)
            stot = big.tile([P, M_CHUNKS], F32)
            nc.vector.scalar_tensor_tensor(
                out=stot[:], in0=rdv[:], scalar=1.0 / EXP8_SCALE, in1=ra[:],
                op0=ALU.mult, op1=ALU.add)
            denom = big.tile([P, M_CHUNKS], F32)
            nc.vector.tensor_scalar_add(denom[:], stot[:],
                                        -float(np.exp(TEMP_SCALE)))
            # ln(denom) via Newton from an analytic seed (keeps ACT on Exp)
            y0 = float(np.log((TWO_N - 1) * np.exp(TEMP_SCALE ** 2 / (2 * D))))
            e1 = float(np.exp(-y0))
            y1 = big.tile([P, M_CHUNKS], F32)
            nc.vector.tensor_scalar(y1[:], denom[:], e1, y0 - 1.0,
                                    ALU.mult, ALU.add)
            e2t = big.tile([P, M_CHUNKS], F32)
            nc.scalar.activation(e2t[:], y1[:], AF.Exp, scale=-1.0)
            tprod = big.tile([P, M_CHUNKS], F32)
            nc.vector.tensor_mul(tprod[:], e2t[:], denom[:])
            lnd = big.tile([P, M_CHUNKS], F32)
            nc.vector.scalar_tensor_tensor(
                out=lnd[:], in0=tprod[:], scalar=-1.0, in1=y1[:],
                op0=ALU.add, op1=ALU.add)
            term = big.tile([P, M_CHUNKS], F32)
            tsum = big.tile([P, 1], F32)
            nc.vector.scalar_tensor_tensor(
                out=term[:], in0=pos[:], scalar=-TEMP_SCALE, in1=lnd[:],
                op0=ALU.mult, op1=ALU.add, accum_out=tsum[:])
            with tc.tile_pool(name="psum2", bufs=1, space="PSUM") as pp2:
                pfin = pp2.tile([1, 1], F32)
                nc.tensor.matmul(pfin[:], ones_sb[:], tsum[:],
                                 start=True, stop=True)
                res = big.tile([1, 1], F32)
                nc.vector.tensor_copy(res[:], pfin[:])
                nc.sync.dma_start(out[:], res[:])

    nc.compile()
    return nc


def _get_nc():
    if "nc" not in _NC_CACHE:
        _NC_CACHE["nc"] = _build_nc()
    return _NC_CACHE["nc"]


def _prepare_in_maps(z_i, z_j):
    import ml_dtypes

    z_full = np.concatenate(
        [np.asarray(z_i, np.float32), np.asarray(z_j, np.float32)], axis=0
    ).astype(ml_dtypes.bfloat16)
    ident = np.eye(P, dtype=np.float32)
    ones = np.ones((P, 1), dtype=np.float32)
    in_maps = []
    for c in range(N_CORES):
        zc = np.roll(z_full, -ROWS_PER_CORE * c, axis=0)
        in_maps.append({"z": np.ascontiguousarray(zc), "ident": ident, "ones": ones})
    return in_maps


def kernel(z_i, z_j):
    nc = _get_nc()
    in_maps = _prepare_in_maps(z_i, z_j)
    res = run_bass_kernel_spmd(nc, in_maps, core_ids=list(range(N_CORES)))
    total = 0.0
    for c in range(N_CORES):
        total += float(res.results[c]["partial"][0, 0])
    loss = total / float(TWO_N)
    return np.float32(loss)


if __name__ == "__main__":
    rng = np.random.default_rng(0)
    z_i = rng.standard_normal((4096, 256), dtype=np.float32)
    z_j = rng.standard_normal((4096, 256), dtype=np.float32)
    print("loss:", kernel(z_i, z_j))


# revision 10
# speedup vs baseline: 1.0145x; 1.0145x over previous
"""NT-Xent loss on 8 Trainium2 NeuronCores — v2.

Baseline (117us) was serialized on two ~55us rooflines: bf16 matmuls on PE
and exp on the ACT engine (1 elem/cycle/lane). v2 attacks both:

  1. fp8e4 DoubleRow matmuls: contract-256 in ONE instruction at ~2x bf16
     stream rate. znT8 [128, 2, 8192] fp8 built on-device; the bf16->fp8
     cast rides a gpsimd (SWDGE) casting DMA — zero compute-engine time.
  2. exp is split between ACT (exact spline exp, accum_out rowsum) and a
     custom fused DVE op EXP8_SUM: Schraudolph-style fast exp done shift-free
     as an 8th power — v = f32(s*C0 + C1) captures round(t*2^20) in the
     2^23 binade (value==bits there), u = v*2^-23 slides the exponent field
     exactly, u^8 = 16*exp(2s)*(1+eps), eps ~ +-30% pointwise sawtooth but
     the C1 tuning zeroes the exp-weighted mean: denominator-sum error
     ~2e-3, loss error ~1e-4 (gate is 2e-2). accum=add fuses the rowsum.
     The stray 16x is divided out in the tail.
  3. Window split: per m-chunk, 4 column windows of 2048; engines assigned
     ~22 ACT / 10 DVE so both run flat out while DVE also owns phase 1
     (sumsq via stt, Newton rsqrt, zn=z*r) which hides under the all-ACT q0.

Per core c: host rotates z by -1024c rows (identical SPMD program); rows
0..1023 are local; positives at +4096; diag handled as constant e^2.
Host sums the 8 scalar partials / 8192.
"""

import os
import sys

sys.path.insert(0, "/opt/trn_rl_repo")
os.environ.setdefault("MYCRO_LOCAL_CACHE", "1")

from operator import add as _op_add

import numpy as np

import concourse.bass as bass
import concourse.mybir as mybir
from concourse import bacc, tile
from concourse.bass_utils import run_bass_kernel_spmd

F32 = mybir.dt.float32
BF16 = mybir.dt.bfloat16
FP8 = mybir.dt.float8e4
AF = mybir.ActivationFunctionType
ALU = mybir.AluOpType
DR = mybir.MatmulPerfMode.DoubleRow

N_CORES = 8
TWO_N = 8192
D = 256
P = 128
NCHUNK = TWO_N // P               # 64 row-chunks of 128
NGROUPS = 8                       # phase-1 pipeline groups
GCHUNK = NCHUNK // NGROUPS        # 8 chunks per group
GROWS = TWO_N // NGROUPS          # 1024 rows per group
ROWS_PER_CORE = TWO_N // N_CORES  # 1024
M_CHUNKS = ROWS_PER_CORE // P     # 8 local row chunks
NCOL = 512                        # matmul free dim (one PSUM bank)
QCOL = 2048                       # consumer window = 4 banks
N_Q = TWO_N // QCOL               # 4
POS_OFF = TWO_N // 2              # 4096
TEMP_SCALE = 2.0                  # 1 / temperature

# ---- EXP8 fast-exp constants (calibrated; see module docstring) ----
_LOG2E = float(np.log2(np.e))
EXP8_C0 = float(np.float32(2.0 * _LOG2E / 8.0 * 2.0**23))
EXP8_C1 = float(np.float32(1.5 * 2.0**23 - 721420.3))
EXP8_C2 = float(np.float32(2.0**-23))
EXP8_SCALE = 16.0                 # u^8 = 16*exp(2s); divided out in tail

# window -> engine: 'A' = ACT exact exp, 'D' = DVE EXP8. q0 all-ACT (DVE
# still finishing phase 1); later qs alternate so both engines stay busy.
_DPAT = {1: (0, 2, 4, 6), 2: (1, 3, 5, 7), 3: (2, 5)}


def _win_engine(m, q):
    return "D" if m in _DPAT.get(q, ()) else "A"


# ---- custom DVE op registration ----
EXP8_NAME = "EXP8_SUM_ANT"


def _exp8_reference(in0, in1, c0, c1, c2):
    p = in0.shape[0]
    v = (in0.astype(np.float32) * np.float32(c0)).astype(np.float32)
    v = (v + np.float32(c1)).astype(np.float32)
    u = (v * np.float32(c2)).astype(np.float32)
    u2 = (u * u).astype(np.float32)
    u4 = (u2 * u2).astype(np.float32)
    u8 = (u4 * u4).astype(np.float32)
    return u8, u8.reshape(p, -1).sum(axis=-1, keepdims=True).astype(np.float32)


def _sqsum_reference(in0, in1, c0, c1, c2):
    p = in0.shape[0]
    x = in0.astype(np.float32)
    b = (x * x).astype(np.float32)
    return b, b.reshape(p, -1).sum(axis=-1, keepdims=True).astype(np.float32)


def _register_op(name, make_spec, perf_en):
    import concourse.dve_ops as dve_ops
    from concourse.dve_spec import _has_src1, lower
    from concourse.dve_uop import DveOpSpec

    for op in dve_ops.OPS:
        if op.name == name:
            return op
    spec = make_spec()
    row = dve_ops._CUSTOM_DVE_ROW_BASE + len(dve_ops.OPS)
    assert row < 0x20, "custom DVE opcode rows exhausted"
    dve_ops._SUB_OPCODE_FOR_NAME[name] = row
    shas = {}
    for ver in ("v3", "v4"):
        try:
            lowered = DveOpSpec(
                name=name, opcode=row, uops=lower(spec, ver=ver),
                rd1_en=_has_src1(spec),
            )
            shas[ver] = lowered.sha(ver)
        except Exception:
            if ver == "v3":
                raise
    op = dve_ops.DveOp(name, spec, subdim=False, uops_sha=shas,
                       perf_en=dict(perf_en))
    dve_ops.OPS.append(op)
    dve_ops.CUSTOM_DVE_SPECS[name] = spec
    return op


def _make_exp8_spec():
    from concourse.dve_spec import C0, C1, C2, Spec, Src0, Zero, sq

    return Spec(
        body=sq(sq(sq((Src0 * C0 + C1) * C2))),
        accum=_op_add,
        accum_init=Zero,
        reference=_exp8_reference,
    )


def _make_sqsum_spec():
    from concourse.dve_spec import Spec, Src0, Zero, sq

    return Spec(
        body=sq(Src0),
        accum=_op_add,
        accum_init=Zero,
        reference=_sqsum_reference,
    )


EXP8_OP = _register_op(EXP8_NAME, _make_exp8_spec, {})
SQSUM_OP = _register_op("SQSUM_ANT", _make_sqsum_spec, {"v3": True})


def _shrink_redundant_ldweights(nc):
    """Consecutive LDWEIGHTS with an identical stationary AP reload data the
    PE already holds. Shrink each repeat to a 1-column reload (idempotent —
    rewrites stationary column 0 with the same bytes) so it costs ~2 cycles
    instead of 256."""
    n = 0
    for f in nc.m.functions:
        for b in f.blocks:
            last_key = None
            for i in b.instructions:
                tn = type(i).__name__
                if tn == "InstLdweights":
                    ap0 = i.ins[0]
                    pairs = [list(p) for p in ap0.ap]
                    key = (ap0.memref, ap0.offset, str(pairs),
                           str(i.perf_mode), str(i.is_transpose))
                    if key == last_key:
                        pairs[-1] = [pairs[-1][0], 1]
                        ap0.ap = pairs
                        i.ins = [ap0]
                        n += 1
                    else:
                        last_key = key
                elif tn in ("InstMatmult", "InstEventSemaphore", "InstDrain",
                            "InstNop"):
                    pass  # none of these disturb the loaded stationary
                elif getattr(i, "engine", None) == mybir.EngineType.PE:
                    last_key = None
    return n

_NC_CACHE = {}


def _build_nc():
    nc = bacc.Bacc(
        "TRN2",
        target_bir_lowering=False,
        debug=False,
        enable_asserts=False,
        num_devices=N_CORES,
    )
    z = nc.dram_tensor("z", [TWO_N, D], BF16, kind="ExternalInput")
    ident = nc.dram_tensor("ident", [P, P], F32, kind="ExternalInput")
    ones = nc.dram_tensor("ones", [P, 1], F32, kind="ExternalInput")
    out = nc.dram_tensor("partial", [1, 1], F32, kind="ExternalOutput")
    zs = nc.dram_tensor("zscratch", [TWO_N, D], BF16, kind="Internal")

    with tile.TileContext(nc) as tc:
        with (
            tc.tile_pool(name="big", bufs=1) as big,
            tc.tile_pool(name="zbpool", bufs=8) as zbpool,
            tc.tile_pool(name="znpool", bufs=4) as znpool,
            tc.tile_pool(name="work", bufs=2) as work,
        ):
            id_sb = big.tile([P, P], F32)
            nc.sync.dma_start(id_sb[:], ident[:])
            ones_sb = big.tile([P, 1], F32)
            nc.sync.dma_start(ones_sb[:], ones[:])

            zt0 = big.tile([P, TWO_N], BF16)
            zt1 = big.tile([P, TWO_N], BF16)
            zts = [zt0, zt1]
            znT8 = big.tile([P, 2, TWO_N], FP8)
            ss = big.tile([P, NCHUNK], F32)
            inv = big.tile([P, NCHUNK], F32)
            sums_a = big.tile([P, M_CHUNKS, N_Q], F32)
            sums_d = big.tile([P, M_CHUNKS, N_Q], F32)
            nc.gpsimd.memset(sums_a[:], 0.0)
            nc.gpsimd.memset(sums_d[:], 0.0)
            pos = big.tile([P, M_CHUNKS], F32)

            zv = z[:].rearrange("(n p) d -> p n d", p=P)
            zsv = zs[:].rearrange("(n p) d -> p n d", p=P)

            # all loads issued up front on the scalar-engine HWDGE ring
            zbg = []
            for g in range(NGROUPS):
                zb = zbpool.tile([P, GCHUNK, D], BF16, tag="zb")
                nc.scalar.dma_start(zb[:], zv[:, g * GCHUNK:(g + 1) * GCHUNK, :])
                zbg.append(zb)

            def sumsq_group(g):
                for c in range(GCHUNK):
                    scr = work.tile([P, D], BF16, tag="sqscr")
                    col = g * GCHUNK + c
                    nc.vector._custom_dve(
                        SQSUM_OP, out=scr[:], in0=zbg[g][:, c, :],
                        accum_out=ss[:, col:col + 1])

            def newton_inv(lo, hi):
                """inv[:, lo:hi] = rsqrt(ss[:, lo:hi]) via 2 Newton steps.
                Runs on the (otherwise idle) gpsimd engine to keep DVE free
                for exp work."""
                w = hi - lo
                rd = 1.0 / float(np.sqrt(D))
                s = ss[:, lo:hi]
                ya = work.tile([P, w], F32, tag="ny0", bufs=2)
                nc.gpsimd.tensor_scalar(ya[:], s, -0.5 / D * rd, 1.5 * rd,
                                        ALU.mult, ALU.add)
                for it in range(2):
                    t1 = work.tile([P, w], F32, tag=f"nt1_{it}", bufs=2)
                    nc.gpsimd.tensor_tensor(t1[:], ya[:], ya[:], ALU.mult)
                    t2 = work.tile([P, w], F32, tag=f"nt2_{it}", bufs=2)
                    nc.gpsimd.tensor_tensor(t2[:], t1[:], s, ALU.mult)
                    t3 = work.tile([P, w], F32, tag=f"nt3_{it}", bufs=2)
                    nc.gpsimd.tensor_scalar(t3[:], t2[:], -0.5, 1.5,
                                            ALU.mult, ALU.add)
                    if it == 1:
                        nc.gpsimd.tensor_tensor(inv[:, lo:hi], ya[:], t3[:],
                                                ALU.mult)
                    else:
                        yb = work.tile([P, w], F32, tag=f"ny{it + 1}", bufs=2)
                        nc.gpsimd.tensor_tensor(yb[:], ya[:], t3[:], ALU.mult)
                        ya = yb

            def finish_group(g):
                """zn = z * inv (DVE), store, transpose, fp8-cast via SWDGE.
                Transposes alternate between the sync and scalar HWDGE rings
                so consecutive groups' transposes overlap."""
                znb = znpool.tile([P, GCHUNK, D], BF16, tag="znb")
                for c in range(GCHUNK):
                    col = g * GCHUNK + c
                    nc.vector.tensor_scalar_mul(
                        znb[:, c, :], zbg[g][:, c, :], inv[:, col:col + 1])
                sl = slice(g * GCHUNK, (g + 1) * GCHUNK)
                nc.sync.dma_start(zsv[:, sl, :], znb[:])
                rsl = slice(g * GROWS, (g + 1) * GROWS)
                nc.sync.dma_start_transpose(zt0[:, rsl], zs[rsl, 0:P])
                nc.sync.dma_start_transpose(zt1[:, rsl], zs[rsl, P:2 * P])
                # bf16 -> fp8e4 dtype-casting DMA (only legal from the
                # gpsimd/SWDGE queue); zero compute-engine cost.
                nc.gpsimd.dma_start(znT8[:, 0, rsl], zt0[:, rsl])
                nc.gpsimd.dma_start(znT8[:, 1, rsl], zt1[:, rsl])
                return znb

            def main_win(m, q):
                """One [128, 2048] window: 4 fp8 DoubleRow matmuls
                (contract 256 in one instruction each), then exp+rowsum on
                ACT (exact) or DVE (EXP8 fast-exp, 16x-scaled)."""
                pt = psum_pool.tile([P, QCOL], F32, tag="sim")
                lhsT = znT8[:, :, m * P:(m + 1) * P]
                col0 = q * QCOL
                for nn in range(QCOL // NCOL):
                    col = col0 + nn * NCOL
                    nc.tensor.matmul(
                        pt[:, nn * NCOL:(nn + 1) * NCOL],
                        lhsT,
                        znT8[:, :, col:col + NCOL],
                        start=True, stop=True, perf_mode=DR,
                    )
                pcol = POS_OFF + m * P
                if col0 <= pcol < col0 + QCOL:
                    off = pcol - col0
                    scr = work.tile([P, P], F32, tag="extr")
                    nc.vector.scalar_tensor_tensor(
                        out=scr[:], in0=pt[:, off:off + P],
                        scalar=1.0, in1=id_sb[:],
                        op0=ALU.mult, op1=ALU.mult,
                        accum_out=pos[:, m:m + 1])
                if _win_engine(m, q) == "A":
                    nc.scalar.activation(
                        pt[:], pt[:], AF.Exp, scale=TEMP_SCALE,
                        accum_out=sums_a[:, m, q:q + 1])
                else:
                    scr8 = work.tile([P, QCOL], BF16, tag="e8out", bufs=2)
                    nc.vector._custom_dve(
                        EXP8_OP, out=scr8[:], in0=pt[:],
                        s0=EXP8_C0, s1=EXP8_C1, imm2=EXP8_C2,
                        accum_out=sums_d[:, m, q:q + 1])

            with tc.tile_pool(name="psum", bufs=2, space="PSUM") as psum_pool:
                # phase 1 fully front-loaded, per-group chains so the stages
                # pipeline across groups on different engines/queues
                znb0 = None
                for g in range(NGROUPS):
                    sumsq_group(g)
                    newton_inv(g * GCHUNK, (g + 1) * GCHUNK)
                    znb = finish_group(g)
                    if g == 0:
                        znb0 = znb
                        # warm-up matmuls bridge the PE HAM window
                        ptw = psum_pool.tile([P, QCOL], F32, tag="sim")
                        for j in range(8):
                            nc.tensor.matmul(
                                ptw[:, (j % 4) * NCOL:(j % 4) * NCOL + NCOL],
                                znb0[:, 0, 0:P],
                                znb0[:, 2 * (j % 4):2 * (j % 4) + 2, :],
                                start=True, stop=True, skip_group_check=True)
                for q in range(N_Q):
                    for m in range(M_CHUNKS):
                        main_win(m, q)

            # ---- tail: per-core partial loss ----
            ra = big.tile([P, M_CHUNKS], F32)
            nc.vector.tensor_reduce(ra[:], sums_a[:],
                                    axis=mybir.AxisListType.X, op=ALU.add)
            rdv = big.tile([P, M_CHUNKS], F32)
            nc.vector.tensor_reduce(rdv[:], sums_d[:],
                                    axis=mybir.AxisListType.X, op=ALU.add)
            stot = big.tile([P, M_CHUNKS], F32)
            nc.vector.scalar_tensor_tensor(
                out=stot[:], in0=rdv[:], scalar=1.0 / EXP8_SCALE, in1=ra[:],
                op0=ALU.mult, op1=ALU.add)
            denom = big.tile([P, M_CHUNKS], F32)
            nc.vector.tensor_scalar_add(denom[:], stot[:],
                                        -float(np.exp(TEMP_SCALE)))
            # ln(denom) via Newton from an analytic seed (keeps ACT on Exp)
            y0 = float(np.log((TWO_N - 1) * np.exp(TEMP_SCALE ** 2 / (2 * D))))
            e1 = float(np.exp(-y0))
            y1 = big.tile([P, M_CHUNKS], F32)
            nc.vector.tensor_scalar(y1[:], denom[:], e1, y0 - 1.0,
                                    ALU.mult, ALU.add)
            e2t = big.tile([P, M_CHUNKS], F32)
            nc.scalar.activation(e2t[:], y1[:], AF.Exp, scale=-1.0)
            tprod = big.tile([P, M_CHUNKS], F32)
            nc.vector.tensor_mul(tprod[:], e2t[:], denom[:])
            lnd = big.tile([P, M_CHUNKS], F32)
            nc.vector.scalar_tensor_tensor(
                out=lnd[:], in0=tprod[:], scalar=-1.0, in1=y1[:],
                op0=ALU.add, op1=ALU.add)
            term = big.tile([P, M_CHUNKS], F32)
            tsum = big.tile([P, 1], F32)
            nc.vector.scalar_tensor_tensor(
                out=term[:], in0=pos[:], scalar=-TEMP_SCALE, in1=lnd[:],
                op0=ALU.mult, op1=ALU.add, accum_out=tsum[:])
            with tc.tile_pool(name="psum2", bufs=1, space="PSUM") as pp2:
                pfin = pp2.tile([1, 1], F32)
                nc.tensor.matmul(pfin[:], ones_sb[:], tsum[:],
                                 start=True, stop=True)
                res = big.tile([1, 1], F32)
                nc.vector.tensor_copy(res[:], pfin[:])
                nc.sync.dma_start(out[:], res[:])

    _shrink_redundant_ldweights(nc)
    nc.compile()
    return nc


def _get_nc():
    if "nc" not in _NC_CACHE:
        _NC_CACHE["nc"] = _build_nc()
    return _NC_CACHE["nc"]


def _prepare_in_maps(z_i, z_j):
    import ml_dtypes

    z_full = np.concatenate(
        [np.asarray(z_i, np.float32), np.asarray(z_j, np.float32)], axis=0
    ).astype(ml_dtypes.bfloat16)
    ident = np.eye(P, dtype=np.float32)
    ones = np.ones((P, 1), dtype=np.float32)
    in_maps = []
    for c in range(N_CORES):
        zc = np.roll(z_full, -ROWS_PER_CORE * c, axis=0)
        in_maps.append({"z": np.ascontiguousarray(zc), "ident": ident, "ones": ones})
    return in_maps


def kernel(z_i, z_j):
    nc = _get_nc()
    in_maps = _prepare_in_maps(z_i, z_j)
    res = run_bass_kernel_spmd(nc, in_maps, core_ids=list(range(N_CORES)))
    total = 0.0
    for c in range(N_CORES):
        total += float(res.results[c]["partial"][0, 0])
    loss = total / float(TWO_N)
    return np.float32(loss)


if __name__ == "__main__":
    rng = np.random.default_rng(0)
    z_i = rng.standard_normal((4096, 256), dtype=np.float32)
    z_j = rng.standard_normal((4096, 256), dtype=np.float32)
    print("loss:", kernel(z_i, z_j))


# revision 17
# speedup vs baseline: 1.0885x; 1.0729x over previous
"""NT-Xent loss on 8 Trainium2 NeuronCores — v2.

Baseline (117us) was serialized on two ~55us rooflines: bf16 matmuls on PE
and exp on the ACT engine (1 elem/cycle/lane). v2 attacks both:

  1. fp8e4 DoubleRow matmuls: contract-256 in ONE instruction at ~2x bf16
     stream rate. znT8 [128, 2, 8192] fp8 built on-device; the bf16->fp8
     cast rides a gpsimd (SWDGE) casting DMA — zero compute-engine time.
  2. exp is split between ACT (exact spline exp, accum_out rowsum) and a
     custom fused DVE op EXP8_SUM: Schraudolph-style fast exp done shift-free
     as an 8th power — v = f32(s*C0 + C1) captures round(t*2^20) in the
     2^23 binade (value==bits there), u = v*2^-23 slides the exponent field
     exactly, u^8 = 16*exp(2s)*(1+eps), eps ~ +-30% pointwise sawtooth but
     the C1 tuning zeroes the exp-weighted mean: denominator-sum error
     ~2e-3, loss error ~1e-4 (gate is 2e-2). accum=add fuses the rowsum.
     The stray 16x is divided out in the tail.
  3. Window split: per m-chunk, 4 column windows of 2048; engines assigned
     ~22 ACT / 10 DVE so both run flat out while DVE also owns phase 1
     (sumsq via stt, Newton rsqrt, zn=z*r) which hides under the all-ACT q0.

Per core c: host rotates z by -1024c rows (identical SPMD program); rows
0..1023 are local; positives at +4096; diag handled as constant e^2.
Host sums the 8 scalar partials / 8192.
"""

import os
import sys

sys.path.insert(0, "/opt/trn_rl_repo")
os.environ.setdefault("MYCRO_LOCAL_CACHE", "1")

from operator import add as _op_add

import numpy as np

import concourse.bass as bass
import concourse.mybir as mybir
from concourse import bacc, tile
from concourse.bass_utils import run_bass_kernel_spmd

F32 = mybir.dt.float32
BF16 = mybir.dt.bfloat16
FP8 = mybir.dt.float8e4
AF = mybir.ActivationFunctionType
ALU = mybir.AluOpType
DR = mybir.MatmulPerfMode.DoubleRow

N_CORES = 8
TWO_N = 8192
D = 256
P = 128
NCHUNK = TWO_N // P               # 64 row-chunks of 128
NGROUPS = 8                       # phase-1 pipeline groups
GCHUNK = NCHUNK // NGROUPS        # 8 chunks per group
GROWS = TWO_N // NGROUPS          # 1024 rows per group
ROWS_PER_CORE = TWO_N // N_CORES  # 1024
M_CHUNKS = ROWS_PER_CORE // P     # 8 local row chunks
NCOL = 512                        # matmul free dim (one PSUM bank)
QCOL = 2048                       # consumer window = 4 banks
N_Q = TWO_N // QCOL               # 4
POS_OFF = TWO_N // 2              # 4096
TEMP_SCALE = 2.0                  # 1 / temperature

# ---- EXP8 fast-exp constants (calibrated; see module docstring) ----
_LOG2E = float(np.log2(np.e))
EXP8_C0 = float(np.float32(2.0 * _LOG2E / 8.0 * 2.0**23))
EXP8_C1 = float(np.float32(1.5 * 2.0**23 - 721420.3))
EXP8_C2 = float(np.float32(2.0**-23))
EXP8_SCALE = 16.0                 # u^8 = 16*exp(2s); divided out in tail

# window -> engine: 'A' = ACT exact exp, 'D' = DVE EXP8. q0 all-ACT (DVE
# still finishing phase 1); later qs alternate so both engines stay busy.
_DPAT = {1: (0, 2, 4, 6), 2: (1, 3, 5, 7), 3: (2, 5)}


def _win_engine(m, q):
    return "D" if m in _DPAT.get(q, ()) else "A"


# ---- custom DVE op registration ----
EXP8_NAME = "EXP8_SUM_ANT"


def _exp8_reference(in0, in1, c0, c1, c2):
    p = in0.shape[0]
    v = (in0.astype(np.float32) * np.float32(c0)).astype(np.float32)
    v = (v + np.float32(c1)).astype(np.float32)
    u = (v * np.float32(c2)).astype(np.float32)
    u2 = (u * u).astype(np.float32)
    u4 = (u2 * u2).astype(np.float32)
    u8 = (u4 * u4).astype(np.float32)
    return u8, u8.reshape(p, -1).sum(axis=-1, keepdims=True).astype(np.float32)


def _sqsum_reference(in0, in1, c0, c1, c2):
    p = in0.shape[0]
    x = in0.astype(np.float32)
    b = (x * x).astype(np.float32)
    return b, b.reshape(p, -1).sum(axis=-1, keepdims=True).astype(np.float32)


def _register_op(name, make_spec, perf_en):
    import concourse.dve_ops as dve_ops
    from concourse.dve_spec import _has_src1, lower
    from concourse.dve_uop import DveOpSpec

    for op in dve_ops.OPS:
        if op.name == name:
            return op
    spec = make_spec()
    row = dve_ops._CUSTOM_DVE_ROW_BASE + len(dve_ops.OPS)
    assert row < 0x20, "custom DVE opcode rows exhausted"
    dve_ops._SUB_OPCODE_FOR_NAME[name] = row
    shas = {}
    for ver in ("v3", "v4"):
        try:
            lowered = DveOpSpec(
                name=name, opcode=row, uops=lower(spec, ver=ver),
                rd1_en=_has_src1(spec),
            )
            shas[ver] = lowered.sha(ver)
        except Exception:
            if ver == "v3":
                raise
    op = dve_ops.DveOp(name, spec, subdim=False, uops_sha=shas,
                       perf_en=dict(perf_en))
    dve_ops.OPS.append(op)
    dve_ops.CUSTOM_DVE_SPECS[name] = spec
    return op


def _make_exp8_spec():
    from concourse.dve_spec import C0, C1, C2, Spec, Src0, Zero, sq

    return Spec(
        body=sq(sq(sq((Src0 * C0 + C1) * C2))),
        accum=_op_add,
        accum_init=Zero,
        reference=_exp8_reference,
    )


def _make_sqsum_spec():
    from concourse.dve_spec import Spec, Src0, Zero, sq

    return Spec(
        body=sq(Src0),
        accum=_op_add,
        accum_init=Zero,
        reference=_sqsum_reference,
    )


def _rsq1_reference(in0, in1, c0, c1, c2):
    s = in0.astype(np.float32)
    y0 = (s * np.float32(c0) + np.float32(c1)).astype(np.float32)
    t = (in1.astype(np.float32) + (s * np.float32(c2)) * (y0 * y0)).astype(np.float32)
    return (y0 * t).astype(np.float32)


def _make_rsq1_spec():
    from concourse.dve_spec import C0, C1, C2, Spec, Src0, Src1, sq

    y0 = Src0 * C0 + C1
    return Spec(
        body=y0 * (Src1 + (Src0 * C2) * sq(y0)),
        reference=_rsq1_reference,
    )


def _rsq2_reference(in0, in1, c0, c1, c2):
    s = in0.astype(np.float32)
    y = in1.astype(np.float32)
    t = (np.float32(c0) + (s * np.float32(c1)) * (y * y)).astype(np.float32)
    return (y * t).astype(np.float32)


def _make_rsq2_spec():
    from concourse.dve_spec import C0, C1, Spec, Src0, Src1, sq

    return Spec(
        body=Src1 * (C0 + (Src0 * C1) * sq(Src1)),
        reference=_rsq2_reference,
    )


EXP8_OP = _register_op(EXP8_NAME, _make_exp8_spec, {})
SQSUM_OP = _register_op("SQSUM_ANT", _make_sqsum_spec, {"v3": True})
RSQ1_OP = _register_op("RSQ1_ANT", _make_rsq1_spec, {})
RSQ2_OP = _register_op("RSQ2_ANT", _make_rsq2_spec, {})


def _shrink_redundant_ldweights(nc):
    """Consecutive LDWEIGHTS with an identical stationary AP reload data the
    PE already holds. Shrink each repeat to a 1-column reload (idempotent —
    rewrites stationary column 0 with the same bytes) so it costs ~2 cycles
    instead of 256."""
    n = 0
    for f in nc.m.functions:
        for b in f.blocks:
            last_key = None
            for i in b.instructions:
                tn = type(i).__name__
                if tn == "InstLdweights":
                    ap0 = i.ins[0]
                    pairs = [list(p) for p in ap0.ap]
                    key = (ap0.memref, ap0.offset, str(pairs),
                           str(i.perf_mode), str(i.is_transpose))
                    if key == last_key:
                        pairs[-1] = [pairs[-1][0], 1]
                        ap0.ap = pairs
                        i.ins = [ap0]
                        n += 1
                    else:
                        last_key = key
                elif tn in ("InstMatmult", "InstEventSemaphore", "InstDrain",
                            "InstNop"):
                    pass  # none of these disturb the loaded stationary
                elif getattr(i, "engine", None) == mybir.EngineType.PE:
                    last_key = None
    return n

_NC_CACHE = {}


def _build_nc():
    nc = bacc.Bacc(
        "TRN2",
        target_bir_lowering=False,
        debug=False,
        enable_asserts=False,
        num_devices=N_CORES,
    )
    z = nc.dram_tensor("z", [TWO_N, D], BF16, kind="ExternalInput")
    ident = nc.dram_tensor("ident", [P, P], F32, kind="ExternalInput")
    ones = nc.dram_tensor("ones", [P, 1], F32, kind="ExternalInput")
    out = nc.dram_tensor("partial", [1, 1], F32, kind="ExternalOutput")
    zs = nc.dram_tensor("zscratch", [TWO_N, D], BF16, kind="Internal")

    with tile.TileContext(nc) as tc:
        with (
            tc.tile_pool(name="big", bufs=1) as big,
            tc.tile_pool(name="zbpool", bufs=8) as zbpool,
            tc.tile_pool(name="znpool", bufs=4) as znpool,
            tc.tile_pool(name="work", bufs=2) as work,
        ):
            id_sb = big.tile([P, P], F32)
            nc.sync.dma_start(id_sb[:], ident[:])
            ones_sb = big.tile([P, 1], F32)
            nc.sync.dma_start(ones_sb[:], ones[:])

            zt0 = big.tile([P, TWO_N], BF16)
            zt1 = big.tile([P, TWO_N], BF16)
            zts = [zt0, zt1]
            znT8 = big.tile([P, 2, TWO_N], FP8)
            ss = big.tile([P, NCHUNK], F32)
            inv = big.tile([P, NCHUNK], F32)
            sums_a = big.tile([P, M_CHUNKS, N_Q], F32)
            sums_d = big.tile([P, M_CHUNKS, N_Q], F32)
            nc.gpsimd.memset(sums_a[:], 0.0)
            nc.gpsimd.memset(sums_d[:], 0.0)
            pos = big.tile([P, M_CHUNKS], F32)

            zv = z[:].rearrange("(n p) d -> p n d", p=P)
            zsv = zs[:].rearrange("(n p) d -> p n d", p=P)

            # all loads issued up front on the scalar-engine HWDGE ring
            zbg = []
            for g in range(NGROUPS):
                zb = zbpool.tile([P, GCHUNK, D], BF16, tag="zb")
                nc.scalar.dma_start(zb[:], zv[:, g * GCHUNK:(g + 1) * GCHUNK, :])
                zbg.append(zb)

            c15 = big.tile([P, GCHUNK], F32)
            nc.vector.memset(c15[:], 1.5)

            def sumsq_group(g):
                """Per-row sum of squares (DVE; Pool rejects accum ops)."""
                for c in range(GCHUNK):
                    scr = work.tile([P, D], BF16, tag="sqscr")
                    col = g * GCHUNK + c
                    nc.vector._custom_dve(
                        SQSUM_OP, out=scr[:], in0=zbg[g][:, c, :],
                        accum_out=ss[:, col:col + 1])

            def newton_inv(g):
                """inv = rsqrt(ss) for group g: 2 fused custom-DVE ops."""
                lo, hi = g * GCHUNK, (g + 1) * GCHUNK
                rd = 1.0 / float(np.sqrt(D))
                s = ss[:, lo:hi]
                y1 = work.tile([P, GCHUNK], F32, tag="ny1", bufs=2)
                nc.vector._custom_dve(
                    RSQ1_OP, out=y1[:], in0=s, in1=c15[:],
                    s0=-0.5 / D * rd, s1=1.5 * rd, imm2=-0.5)
                nc.vector._custom_dve(
                    RSQ2_OP, out=inv[:, lo:hi], in0=s, in1=y1[:],
                    s0=1.5, s1=-0.5)

            def finish_group(g):
                """zn = z * inv, chunks split DVE / ACT (per-partition AP
                scalars are not legal on the Pool engine)."""
                znb = znpool.tile([P, GCHUNK, D], BF16, tag="znb")
                for c in range(GCHUNK):
                    col = g * GCHUNK + c
                    if c >= 5:
                        nc.scalar.activation(
                            znb[:, c, :], zbg[g][:, c, :], AF.Copy,
                            scale=inv[:, col:col + 1])
                    else:
                        nc.vector.tensor_scalar_mul(
                            znb[:, c, :], zbg[g][:, c, :], inv[:, col:col + 1])
                sl = slice(g * GCHUNK, (g + 1) * GCHUNK)
                nc.sync.dma_start(zsv[:, sl, :], znb[:])
                return znb

            def transpose_pair(k):
                """DMA-transpose row-pair [2048 rows] of the zn scratch."""
                rsl = slice(k * 2 * GROWS, (k + 1) * 2 * GROWS)
                nc.sync.dma_start_transpose(zt0[:, rsl], zs[rsl, 0:P])
                nc.sync.dma_start_transpose(zt1[:, rsl], zs[rsl, P:2 * P])

            def cast_pair(k):
                """bf16 -> fp8e4 casting DMA (gpsimd/SWDGE only)."""
                rsl = slice(k * 2 * GROWS, (k + 1) * 2 * GROWS)
                nc.gpsimd.dma_start(znT8[:, 0, rsl], zt0[:, rsl])
                nc.gpsimd.dma_start(znT8[:, 1, rsl], zt1[:, rsl])

            def main_win(m, q):
                """One [128, 2048] window: 4 fp8 DoubleRow matmuls
                (contract 256 in one instruction each), then exp+rowsum on
                ACT (exact) or DVE (EXP8 fast-exp, 16x-scaled)."""
                pt = psum_pool.tile([P, QCOL], F32, tag="sim")
                lhsT = znT8[:, :, m * P:(m + 1) * P]
                col0 = q * QCOL
                for nn in range(QCOL // NCOL):
                    col = col0 + nn * NCOL
                    nc.tensor.matmul(
                        pt[:, nn * NCOL:(nn + 1) * NCOL],
                        lhsT,
                        znT8[:, :, col:col + NCOL],
                        start=True, stop=True, perf_mode=DR,
                    )
                pcol = POS_OFF + m * P
                if col0 <= pcol < col0 + QCOL:
                    off = pcol - col0
                    scr = work.tile([P, P], F32, tag="extr")
                    nc.vector.scalar_tensor_tensor(
                        out=scr[:], in0=pt[:, off:off + P],
                        scalar=1.0, in1=id_sb[:],
                        op0=ALU.mult, op1=ALU.mult,
                        accum_out=pos[:, m:m + 1])
                if _win_engine(m, q) == "A":
                    nc.scalar.activation(
                        pt[:], pt[:], AF.Exp, scale=TEMP_SCALE,
                        accum_out=sums_a[:, m, q:q + 1])
                else:
                    scr8 = work.tile([P, QCOL], BF16, tag="e8out", bufs=2)
                    nc.vector._custom_dve(
                        EXP8_OP, out=scr8[:], in0=pt[:],
                        s0=EXP8_C0, s1=EXP8_C1, imm2=EXP8_C2,
                        accum_out=sums_d[:, m, q:q + 1])

            with tc.tile_pool(name="psum", bufs=2, space="PSUM") as psum_pool:
                # phase 1 fully front-loaded; per-group chains pipeline across
                # DVE/gpsimd/ACT. Casts are emitted one pair late so the Pool
                # queue never stalls waiting on a transpose. Dummy matmuls
                # keep the PE HAM clock-gate warm through phase 1.
                ptw = psum_pool.tile([P, QCOL], F32, tag="sim")
                nwarm = [0]

                def keep_warm(n, g):
                    # dummy matmuls gated on group g's load: spaced ~load
                    # cadence, they hold the PE HAM gate at 2.4 GHz until
                    # the real window matmuls start flowing
                    for j in range(n):
                        w = nwarm[0] % 4
                        nc.tensor.matmul(
                            ptw[:, w * NCOL:w * NCOL + NCOL],
                            zbg[g][:, 0, 0:P],
                            zbg[g][:, 2 * (w % 4):2 * (w % 4) + 2, :],
                            start=True, stop=True, skip_group_check=True)
                        nwarm[0] += 1

                for g in range(NGROUPS):
                    sumsq_group(g)
                    newton_inv(g)
                    finish_group(g)
                    if g < 3:
                        keep_warm(4 if g == 0 else 2, g)
                    if g % 2 == 1:
                        transpose_pair(g // 2)
                    if g >= 3 and g % 2 == 1:
                        cast_pair(g // 2 - 1)
                cast_pair(NGROUPS // 2 - 1)
                for q in range(N_Q):
                    for m in range(M_CHUNKS):
                        main_win(m, q)

            # ---- tail: per-core partial loss ----
            ra = big.tile([P, M_CHUNKS], F32)
            nc.vector.tensor_reduce(ra[:], sums_a[:],
                                    axis=mybir.AxisListType.X, op=ALU.add)
            rdv = big.tile([P, M_CHUNKS], F32)
            nc.vector.tensor_reduce(rdv[:], sums_d[:],
                                    axis=mybir.AxisListType.X, op=ALU.add)
            stot = big.tile([P, M_CHUNKS], F32)
            nc.vector.scalar_tensor_tensor(
                out=stot[:], in0=rdv[:], scalar=1.0 / EXP8_SCALE, in1=ra[:],
                op0=ALU.mult, op1=ALU.add)
            denom = big.tile([P, M_CHUNKS], F32)
            nc.vector.tensor_scalar_add(denom[:], stot[:],
                                        -float(np.exp(TEMP_SCALE)))
            # ln(denom) via Newton from an analytic seed (keeps ACT on Exp)
            y0 = float(np.log((TWO_N - 1) * np.exp(TEMP_SCALE ** 2 / (2 * D))))
            e1 = float(np.exp(-y0))
            y1 = big.tile([P, M_CHUNKS], F32)
            nc.vector.tensor_scalar(y1[:], denom[:], e1, y0 - 1.0,
                                    ALU.mult, ALU.add)
            e2t = big.tile([P, M_CHUNKS], F32)
            nc.scalar.activation(e2t[:], y1[:], AF.Exp, scale=-1.0)
            tprod = big.tile([P, M_CHUNKS], F32)
            nc.vector.tensor_mul(tprod[:], e2t[:], denom[:])
            lnd = big.tile([P, M_CHUNKS], F32)
            nc.vector.scalar_tensor_tensor(
                out=lnd[:], in0=tprod[:], scalar=-1.0, in1=y1[:],
                op0=ALU.add, op1=ALU.add)
            term = big.tile([P, M_CHUNKS], F32)
            tsum = big.tile([P, 1], F32)
            nc.vector.scalar_tensor_tensor(
                out=term[:], in0=pos[:], scalar=-TEMP_SCALE, in1=lnd[:],
                op0=ALU.mult, op1=ALU.add, accum_out=tsum[:])
            with tc.tile_pool(name="psum2", bufs=1, space="PSUM") as pp2:
                pfin = pp2.tile([1, 1], F32)
                nc.tensor.matmul(pfin[:], ones_sb[:], tsum[:],
                                 start=True, stop=True)
                res = big.tile([1, 1], F32)
                nc.vector.tensor_copy(res[:], pfin[:])
                nc.sync.dma_start(out[:], res[:])

    _shrink_redundant_ldweights(nc)
    nc.compile()
    return nc


def _get_nc():
    if "nc" not in _NC_CACHE:
        _NC_CACHE["nc"] = _build_nc()
    return _NC_CACHE["nc"]


def _prepare_in_maps(z_i, z_j):
    import ml_dtypes

    z_full = np.concatenate(
        [np.asarray(z_i, np.float32), np.asarray(z_j, np.float32)], axis=0
    ).astype(ml_dtypes.bfloat16)
    ident = np.eye(P, dtype=np.float32)
    ones = np.ones((P, 1), dtype=np.float32)
    in_maps = []
    for c in range(N_CORES):
        zc = np.roll(z_full, -ROWS_PER_CORE * c, axis=0)
        in_maps.append({"z": np.ascontiguousarray(zc), "ident": ident, "ones": ones})
    return in_maps


def kernel(z_i, z_j):
    nc = _get_nc()
    in_maps = _prepare_in_maps(z_i, z_j)
    res = run_bass_kernel_spmd(nc, in_maps, core_ids=list(range(N_CORES)))
    total = 0.0
    for c in range(N_CORES):
        total += float(res.results[c]["partial"][0, 0])
    loss = total / float(TWO_N)
    return np.float32(loss)


if __name__ == "__main__":
    rng = np.random.default_rng(0)
    z_i = rng.standard_normal((4096, 256), dtype=np.float32)
    z_j = rng.standard_normal((4096, 256), dtype=np.float32)
    print("loss:", kernel(z_i, z_j))


# revision 20
# speedup vs baseline: 1.1456x; 1.0525x over previous
"""NT-Xent loss on 8 Trainium2 NeuronCores — v2.

Baseline (117us) was serialized on two ~55us rooflines: bf16 matmuls on PE
and exp on the ACT engine (1 elem/cycle/lane). v2 attacks both:

  1. fp8e4 DoubleRow matmuls: contract-256 in ONE instruction at ~2x bf16
     stream rate. znT8 [128, 2, 8192] fp8 built on-device; the bf16->fp8
     cast rides a gpsimd (SWDGE) casting DMA — zero compute-engine time.
  2. exp is split between ACT (exact spline exp, accum_out rowsum) and a
     custom fused DVE op EXP8_SUM: Schraudolph-style fast exp done shift-free
     as an 8th power — v = f32(s*C0 + C1) captures round(t*2^20) in the
     2^23 binade (value==bits there), u = v*2^-23 slides the exponent field
     exactly, u^8 = 16*exp(2s)*(1+eps), eps ~ +-30% pointwise sawtooth but
     the C1 tuning zeroes the exp-weighted mean: denominator-sum error
     ~2e-3, loss error ~1e-4 (gate is 2e-2). accum=add fuses the rowsum.
     The stray 16x is divided out in the tail.
  3. Window split: per m-chunk, 4 column windows of 2048; engines assigned
     ~22 ACT / 10 DVE so both run flat out while DVE also owns phase 1
     (sumsq via stt, Newton rsqrt, zn=z*r) which hides under the all-ACT q0.

Per core c: host rotates z by -1024c rows (identical SPMD program); rows
0..1023 are local; positives at +4096; diag handled as constant e^2.
Host sums the 8 scalar partials / 8192.
"""

import os
import sys

sys.path.insert(0, "/opt/trn_rl_repo")
os.environ.setdefault("MYCRO_LOCAL_CACHE", "1")

from operator import add as _op_add

import numpy as np

import concourse.bass as bass
import concourse.mybir as mybir
from concourse import bacc, tile
from concourse.bass_utils import run_bass_kernel_spmd

F32 = mybir.dt.float32
BF16 = mybir.dt.bfloat16
FP8 = mybir.dt.float8e4
AF = mybir.ActivationFunctionType
ALU = mybir.AluOpType
DR = mybir.MatmulPerfMode.DoubleRow

N_CORES = 8
TWO_N = 8192
D = 256
P = 128
NCHUNK = TWO_N // P               # 64 row-chunks of 128
NGROUPS = 8                       # phase-1 pipeline groups
GCHUNK = NCHUNK // NGROUPS        # 8 chunks per group
GROWS = TWO_N // NGROUPS          # 1024 rows per group
ROWS_PER_CORE = TWO_N // N_CORES  # 1024
M_CHUNKS = ROWS_PER_CORE // P     # 8 local row chunks
NCOL = 512                        # matmul free dim (one PSUM bank)
QCOL = 2048                       # consumer window = 4 banks
N_Q = TWO_N // QCOL               # 4
POS_OFF = TWO_N // 2              # 4096
TEMP_SCALE = 2.0                  # 1 / temperature

# ---- EXP8 fast-exp constants (calibrated; see module docstring) ----
_LOG2E = float(np.log2(np.e))
EXP8_C0 = float(np.float32(2.0 * _LOG2E / 8.0 * 2.0**23))
EXP8_C1 = float(np.float32(1.5 * 2.0**23 - 721420.3))
EXP8_C2 = float(np.float32(2.0**-23))
EXP8_SCALE = 16.0                 # u^8 = 16*exp(2s); divided out in tail

# window -> engine: 'A' = ACT exact exp, 'D' = DVE EXP8. q0/q1 all-ACT —
# DVE runs the whole of phase 1 uninterrupted underneath; the DVE windows
# live late (q2/q3) where phase 1 is done and ACT is the scarce engine.
_DPAT = {2: (0, 2, 4, 6), 3: (0, 1, 3, 4, 6, 7)}


def _win_engine(m, q):
    return "D" if m in _DPAT.get(q, ()) else "A"


# ---- custom DVE op registration ----
EXP8_NAME = "EXP8_SUM_ANT"


def _exp8_reference(in0, in1, c0, c1, c2):
    p = in0.shape[0]
    v = (in0.astype(np.float32) * np.float32(c0)).astype(np.float32)
    v = (v + np.float32(c1)).astype(np.float32)
    u = (v * np.float32(c2)).astype(np.float32)
    u2 = (u * u).astype(np.float32)
    u4 = (u2 * u2).astype(np.float32)
    u8 = (u4 * u4).astype(np.float32)
    return u8, u8.reshape(p, -1).sum(axis=-1, keepdims=True).astype(np.float32)


def _sqsum_reference(in0, in1, c0, c1, c2):
    p = in0.shape[0]
    x = in0.astype(np.float32)
    b = (x * x).astype(np.float32)
    return b, b.reshape(p, -1).sum(axis=-1, keepdims=True).astype(np.float32)


def _register_op(name, make_spec, perf_en):
    import concourse.dve_ops as dve_ops
    from concourse.dve_spec import _has_src1, lower
    from concourse.dve_uop import DveOpSpec

    for op in dve_ops.OPS:
        if op.name == name:
            return op
    spec = make_spec()
    row = dve_ops._CUSTOM_DVE_ROW_BASE + len(dve_ops.OPS)
    assert row < 0x20, "custom DVE opcode rows exhausted"
    dve_ops._SUB_OPCODE_FOR_NAME[name] = row
    shas = {}
    for ver in ("v3", "v4"):
        try:
            lowered = DveOpSpec(
                name=name, opcode=row, uops=lower(spec, ver=ver),
                rd1_en=_has_src1(spec),
            )
            shas[ver] = lowered.sha(ver)
        except Exception:
            if ver == "v3":
                raise
    op = dve_ops.DveOp(name, spec, subdim=False, uops_sha=shas,
                       perf_en=dict(perf_en))
    dve_ops.OPS.append(op)
    dve_ops.CUSTOM_DVE_SPECS[name] = spec
    return op


def _make_exp8_spec():
    from concourse.dve_spec import C0, C1, C2, Spec, Src0, Zero, sq

    return Spec(
        body=sq(sq(sq((Src0 * C0 + C1) * C2))),
        accum=_op_add,
        accum_init=Zero,
        reference=_exp8_reference,
    )


def _make_sqsum_spec():
    from concourse.dve_spec import Spec, Src0, Zero, sq

    return Spec(
        body=sq(Src0),
        accum=_op_add,
        accum_init=Zero,
        reference=_sqsum_reference,
    )


def _rsq1_reference(in0, in1, c0, c1, c2):
    s = in0.astype(np.float32)
    y0 = (s * np.float32(c0) + np.float32(c1)).astype(np.float32)
    t = (in1.astype(np.float32) + (s * np.float32(c2)) * (y0 * y0)).astype(np.float32)
    return (y0 * t).astype(np.float32)


def _make_rsq1_spec():
    from concourse.dve_spec import C0, C1, C2, Spec, Src0, Src1, sq

    y0 = Src0 * C0 + C1
    return Spec(
        body=y0 * (Src1 + (Src0 * C2) * sq(y0)),
        reference=_rsq1_reference,
    )


def _rsq2_reference(in0, in1, c0, c1, c2):
    s = in0.astype(np.float32)
    y = in1.astype(np.float32)
    t = (np.float32(c0) + (s * np.float32(c1)) * (y * y)).astype(np.float32)
    return (y * t).astype(np.float32)


def _make_rsq2_spec():
    from concourse.dve_spec import C0, C1, Spec, Src0, Src1, sq

    return Spec(
        body=Src1 * (C0 + (Src0 * C1) * sq(Src1)),
        reference=_rsq2_reference,
    )


EXP8_OP = _register_op(EXP8_NAME, _make_exp8_spec, {})
SQSUM_OP = _register_op("SQSUM_ANT", _make_sqsum_spec, {"v3": True})
RSQ1_OP = _register_op("RSQ1_ANT", _make_rsq1_spec, {})
RSQ2_OP = _register_op("RSQ2_ANT", _make_rsq2_spec, {})


def _shrink_redundant_ldweights(nc):
    """Consecutive LDWEIGHTS with an identical stationary AP reload data the
    PE already holds. Shrink each repeat to a 1-column reload (idempotent —
    rewrites stationary column 0 with the same bytes) so it costs ~2 cycles
    instead of 256."""
    n = 0
    for f in nc.m.functions:
        for b in f.blocks:
            last_key = None
            for i in b.instructions:
                tn = type(i).__name__
                if tn == "InstLdweights":
                    ap0 = i.ins[0]
                    pairs = [list(p) for p in ap0.ap]
                    key = (ap0.memref, ap0.offset, str(pairs),
                           str(i.perf_mode), str(i.is_transpose))
                    if key == last_key:
                        pairs[-1] = [pairs[-1][0], 1]
                        ap0.ap = pairs
                        i.ins = [ap0]
                        n += 1
                    else:
                        last_key = key
                elif tn in ("InstMatmult", "InstEventSemaphore", "InstDrain",
                            "InstNop"):
                    pass  # none of these disturb the loaded stationary
                elif getattr(i, "engine", None) == mybir.EngineType.PE:
                    last_key = None
    return n

_NC_CACHE = {}


def _build_nc():
    nc = bacc.Bacc(
        "TRN2",
        target_bir_lowering=False,
        debug=False,
        enable_asserts=False,
        num_devices=N_CORES,
    )
    z = nc.dram_tensor("z", [TWO_N, D], BF16, kind="ExternalInput")
    ident = nc.dram_tensor("ident", [P, P], F32, kind="ExternalInput")
    ones = nc.dram_tensor("ones", [P, 1], F32, kind="ExternalInput")
    out = nc.dram_tensor("partial", [1, 1], F32, kind="ExternalOutput")
    zs = nc.dram_tensor("zscratch", [TWO_N, D], BF16, kind="Internal")

    with tile.TileContext(nc) as tc:
        with (
            tc.tile_pool(name="big", bufs=1) as big,
            tc.tile_pool(name="zbpool", bufs=8) as zbpool,
            tc.tile_pool(name="znpool", bufs=4) as znpool,
            tc.tile_pool(name="work", bufs=2) as work,
        ):
            id_sb = big.tile([P, P], F32)
            nc.sync.dma_start(id_sb[:], ident[:])
            ones_sb = big.tile([P, 1], F32)
            nc.sync.dma_start(ones_sb[:], ones[:])

            zt0 = big.tile([P, TWO_N], BF16)
            zt1 = big.tile([P, TWO_N], BF16)
            zts = [zt0, zt1]
            znT8 = big.tile([P, 2, TWO_N], FP8)
            ss = big.tile([P, NCHUNK], F32)
            inv = big.tile([P, NCHUNK], F32)
            sums_a = big.tile([P, M_CHUNKS, N_Q], F32)
            sums_d = big.tile([P, M_CHUNKS, N_Q], F32)
            nc.gpsimd.memset(sums_a[:], 0.0)
            nc.gpsimd.memset(sums_d[:], 0.0)
            pos = big.tile([P, M_CHUNKS], F32)

            zv = z[:].rearrange("(n p) d -> p n d", p=P)
            zsv = zs[:].rearrange("(n p) d -> p n d", p=P)

            # all loads issued up front on the scalar-engine HWDGE ring
            zbg = []
            for g in range(NGROUPS):
                zb = zbpool.tile([P, GCHUNK, D], BF16, tag="zb")
                nc.scalar.dma_start(zb[:], zv[:, g * GCHUNK:(g + 1) * GCHUNK, :])
                zbg.append(zb)

            c15 = big.tile([P, GCHUNK], F32)
            nc.vector.memset(c15[:], 1.5)

            def sumsq_group(g):
                """Per-row sum of squares (DVE; Pool rejects accum ops)."""
                for c in range(GCHUNK):
                    scr = work.tile([P, D], BF16, tag="sqscr")
                    col = g * GCHUNK + c
                    nc.vector._custom_dve(
                        SQSUM_OP, out=scr[:], in0=zbg[g][:, c, :],
                        accum_out=ss[:, col:col + 1])

            def newton_inv(g):
                """inv = rsqrt(ss) for group g: 2 fused custom-DVE ops."""
                lo, hi = g * GCHUNK, (g + 1) * GCHUNK
                rd = 1.0 / float(np.sqrt(D))
                s = ss[:, lo:hi]
                y1 = work.tile([P, GCHUNK], F32, tag="ny1", bufs=2)
                nc.vector._custom_dve(
                    RSQ1_OP, out=y1[:], in0=s, in1=c15[:],
                    s0=-0.5 / D * rd, s1=1.5 * rd, imm2=-0.5)
                nc.vector._custom_dve(
                    RSQ2_OP, out=inv[:, lo:hi], in0=s, in1=y1[:],
                    s0=1.5, s1=-0.5)

            def finish_group(g):
                """zn = z * inv, chunks split DVE / ACT (per-partition AP
                scalars are not legal on the Pool engine)."""
                znb = znpool.tile([P, GCHUNK, D], BF16, tag="znb")
                for c in range(GCHUNK):
                    col = g * GCHUNK + c
                    if c >= 3:
                        nc.scalar.activation(
                            znb[:, c, :], zbg[g][:, c, :], AF.Copy,
                            scale=inv[:, col:col + 1])
                    else:
                        nc.vector.tensor_scalar_mul(
                            znb[:, c, :], zbg[g][:, c, :], inv[:, col:col + 1])
                sl = slice(g * GCHUNK, (g + 1) * GCHUNK)
                nc.sync.dma_start(zsv[:, sl, :], znb[:])
                return znb

            def transpose_pair(k):
                """DMA-transpose row-pair [2048 rows] of the zn scratch."""
                rsl = slice(k * 2 * GROWS, (k + 1) * 2 * GROWS)
                nc.sync.dma_start_transpose(zt0[:, rsl], zs[rsl, 0:P])
                nc.sync.dma_start_transpose(zt1[:, rsl], zs[rsl, P:2 * P])

            def cast_pair(k):
                """bf16 -> fp8e4 casting DMA (gpsimd/SWDGE only)."""
                rsl = slice(k * 2 * GROWS, (k + 1) * 2 * GROWS)
                nc.gpsimd.dma_start(znT8[:, 0, rsl], zt0[:, rsl])
                nc.gpsimd.dma_start(znT8[:, 1, rsl], zt1[:, rsl])

            def main_win(m, q):
                """One [128, 2048] window: 4 fp8 DoubleRow matmuls
                (contract 256 in one instruction each), then exp+rowsum on
                ACT (exact) or DVE (EXP8 fast-exp, 16x-scaled)."""
                pt = psum_pool.tile([P, QCOL], F32, tag="sim")
                lhsT = znT8[:, :, m * P:(m + 1) * P]
                col0 = q * QCOL
                for nn in range(QCOL // NCOL):
                    col = col0 + nn * NCOL
                    nc.tensor.matmul(
                        pt[:, nn * NCOL:(nn + 1) * NCOL],
                        lhsT,
                        znT8[:, :, col:col + NCOL],
                        start=True, stop=True, perf_mode=DR,
                    )
                pcol = POS_OFF + m * P
                if col0 <= pcol < col0 + QCOL:
                    off = pcol - col0
                    scr = work.tile([P, P], F32, tag="extr")
                    nc.vector.scalar_tensor_tensor(
                        out=scr[:], in0=pt[:, off:off + P],
                        scalar=1.0, in1=id_sb[:],
                        op0=ALU.mult, op1=ALU.mult,
                        accum_out=pos[:, m:m + 1])
                if _win_engine(m, q) == "A":
                    nc.scalar.activation(
                        pt[:], pt[:], AF.Exp, scale=TEMP_SCALE,
                        accum_out=sums_a[:, m, q:q + 1])
                else:
                    scr8 = work.tile([P, QCOL], BF16, tag="e8out", bufs=2)
                    nc.vector._custom_dve(
                        EXP8_OP, out=scr8[:], in0=pt[:],
                        s0=EXP8_C0, s1=EXP8_C1, imm2=EXP8_C2,
                        accum_out=sums_d[:, m, q:q + 1])

            with tc.tile_pool(name="psum", bufs=2, space="PSUM") as psum_pool:
                # phase 1 fully front-loaded; per-group chains pipeline across
                # DVE/gpsimd/ACT. Casts are emitted one pair late so the Pool
                # queue never stalls waiting on a transpose. Dummy matmuls
                # keep the PE HAM clock-gate warm through phase 1.
                ptw = psum_pool.tile([P, QCOL], F32, tag="sim")
                nwarm = [0]

                def keep_warm(n, g):
                    # dummy matmuls gated on group g's load: spaced ~load
                    # cadence, they hold the PE HAM gate at 2.4 GHz until
                    # the real window matmuls start flowing
                    for j in range(n):
                        w = nwarm[0] % 4
                        nc.tensor.matmul(
                            ptw[:, w * NCOL:w * NCOL + NCOL],
                            zbg[g][:, 0, 0:P],
                            zbg[g][:, 2 * (w % 4):2 * (w % 4) + 2, :],
                            start=True, stop=True, skip_group_check=True)
                        nwarm[0] += 1

                for g in range(NGROUPS):
                    sumsq_group(g)
                    newton_inv(g)
                    finish_group(g)
                    if g < 3:
                        keep_warm(4 if g == 0 else 2, g)
                    if g % 2 == 1:
                        # the Pool queue only carries casts now — no need to
                        # delay them; fire as soon as the transposes land
                        transpose_pair(g // 2)
                        cast_pair(g // 2)
                for q in range(N_Q):
                    for m in range(M_CHUNKS):
                        main_win(m, q)

            # ---- tail: per-core partial loss ----
            ra = big.tile([P, M_CHUNKS], F32)
            nc.vector.tensor_reduce(ra[:], sums_a[:],
                                    axis=mybir.AxisListType.X, op=ALU.add)
            rdv = big.tile([P, M_CHUNKS], F32)
            nc.vector.tensor_reduce(rdv[:], sums_d[:],
                                    axis=mybir.AxisListType.X, op=ALU.add)
            stot = big.tile([P, M_CHUNKS], F32)
            nc.vector.scalar_tensor_tensor(
                out=stot[:], in0=rdv[:], scalar=1.0 / EXP8_SCALE, in1=ra[:],
                op0=ALU.mult, op1=ALU.add)
            denom = big.tile([P, M_CHUNKS], F32)
            nc.vector.tensor_scalar_add(denom[:], stot[:],
                                        -float(np.exp(TEMP_SCALE)))
            # ln(denom) via Newton from an analytic seed (keeps ACT on Exp)
            y0 = float(np.log((TWO_N - 1) * np.exp(TEMP_SCALE ** 2 / (2 * D))))
            e1 = float(np.exp(-y0))
            y1 = big.tile([P, M_CHUNKS], F32)
            nc.vector.tensor_scalar(y1[:], denom[:], e1, y0 - 1.0,
                                    ALU.mult, ALU.add)
            e2t = big.tile([P, M_CHUNKS], F32)
            nc.scalar.activation(e2t[:], y1[:], AF.Exp, scale=-1.0)
            tprod = big.tile([P, M_CHUNKS], F32)
            nc.vector.tensor_mul(tprod[:], e2t[:], denom[:])
            lnd = big.tile([P, M_CHUNKS], F32)
            nc.vector.scalar_tensor_tensor(
                out=lnd[:], in0=tprod[:], scalar=-1.0, in1=y1[:],
                op0=ALU.add, op1=ALU.add)
            term = big.tile([P, M_CHUNKS], F32)
            tsum = big.tile([P, 1], F32)
            nc.vector.scalar_tensor_tensor(
                out=term[:], in0=pos[:], scalar=-TEMP_SCALE, in1=lnd[:],
                op0=ALU.mult, op1=ALU.add, accum_out=tsum[:])
            with tc.tile_pool(name="psum2", bufs=1, space="PSUM") as pp2:
                pfin = pp2.tile([1, 1], F32)
                nc.tensor.matmul(pfin[:], ones_sb[:], tsum[:],
                                 start=True, stop=True)
                res = big.tile([1, 1], F32)
                nc.vector.tensor_copy(res[:], pfin[:])
                nc.sync.dma_start(out[:], res[:])

    _shrink_redundant_ldweights(nc)
    nc.compile()
    return nc


def _get_nc():
    if "nc" not in _NC_CACHE:
        _NC_CACHE["nc"] = _build_nc()
    return _NC_CACHE["nc"]


def _prepare_in_maps(z_i, z_j):
    import ml_dtypes

    z_full = np.concatenate(
        [np.asarray(z_i, np.float32), np.asarray(z_j, np.float32)], axis=0
    ).astype(ml_dtypes.bfloat16)
    ident = np.eye(P, dtype=np.float32)
    ones = np.ones((P, 1), dtype=np.float32)
    in_maps = []
    for c in range(N_CORES):
        zc = np.roll(z_full, -ROWS_PER_CORE * c, axis=0)
        in_maps.append({"z": np.ascontiguousarray(zc), "ident": ident, "ones": ones})
    return in_maps


def kernel(z_i, z_j):
    nc = _get_nc()
    in_maps = _prepare_in_maps(z_i, z_j)
    res = run_bass_kernel_spmd(nc, in_maps, core_ids=list(range(N_CORES)))
    total = 0.0
    for c in range(N_CORES):
        total += float(res.results[c]["partial"][0, 0])
    loss = total / float(TWO_N)
    return np.float32(loss)


if __name__ == "__main__":
    rng = np.random.default_rng(0)
    z_i = rng.standard_normal((4096, 256), dtype=np.float32)
    z_j = rng.standard_normal((4096, 256), dtype=np.float32)
    print("loss:", kernel(z_i, z_j))
